# revision 28
# baseline (speedup 1.0000x reference)
"""GCN layer (DGL GraphConv norm='both' + relu + residual + LayerNorm) on 8 trn2 NeuronCores.

Final: gathers run directly on an fp16 cast of the node features (no device-side
table build), with the src-degree norm folded into the one-hot scatter masks.
One batched gather per dst block via gpsimd.dma_gather (Q7 'mlp' library) on a
pair-row view [npair, 128] so indices fit int16 (idx = src//2). Each 128-edge
tile runs two masked matmuls — lhsT slices the even/odd half of the gathered
pair row against per-parity masks ind_e/ind_o — so no parity sorting is needed
and the tile schedule is minimal (K[j] = max over cores of ceil(edges/128)).
Dst-degree norm vectors are precomputed host-side. The kernel is bound by Q7
descriptor generation at ~7.6ns per gathered row; everything else hides under it.
"""

import numpy as np


def _ensure_path():
    try:
        import concourse  # noqa: F401
    except ImportError:
        import sys

        for p in ("/opt/trn_rl_repo", "/root/.axon_site/_ro/trn_rl_repo"):
            if p not in sys.path:
                sys.path.insert(0, p)


P = 128
LN_EPS = 1e-5


# ---------------------------------------------------------------- host prep
def host_prep(feats, src, dst, W, b, gamma, beta, n_cores):
    N, D = feats.shape
    assert N % n_cores == 0 and N % 2 == 0
    npc = N // n_cores                      # nodes per core
    nblk = (npc + P - 1) // P               # 128-node blocks per core
    rows_pp = (N + 2 + P - 1) // P          # table rows per partition
    npad = rows_pp * P                      # padded table rows (even, >= N+2)
    npair = npad // 2
    zero_pair = N // 2                      # nodes N, N+1 are zero rows

    src = np.asarray(src).astype(np.int64)
    dst = np.asarray(dst).astype(np.int64)

    feats16_pad = np.zeros((npad, D), np.float16)
    feats16_pad[:N] = feats.astype(np.float16)

    order = np.argsort(dst, kind="stable")
    src_s = src[order]
    dst_s = dst[order]
    rp_dst = np.searchsorted(dst_s, np.arange(N + 1)).astype(np.int64)

    # host-side degree vectors
    deg_out = np.bincount(src, minlength=N).astype(np.float64)
    deg_in = np.bincount(dst, minlength=N).astype(np.float64)
    dgo_val = (1.0 / np.sqrt(np.clip(deg_out, 1.0, None))).astype(np.float32)
    din_val = (1.0 / np.sqrt(np.clip(deg_in, 1.0, None))).astype(np.float32)

    # ---- per (core, block) edge lists (no parity sort: each tile runs two
    # masked matmuls, one per src-parity half of the gathered pair) ---------
    nt = np.zeros((n_cores, nblk), np.int64)
    blk_edges = [[None] * nblk for _ in range(n_cores)]
    for m in range(n_cores):
        base = m * npc
        for j in range(nblk):
            lo = rp_dst[base + min(j * P, npc)]
            hi = rp_dst[base + min((j + 1) * P, npc)]
            blk_edges[m][j] = (src_s[lo:hi], dst_s[lo:hi] - (base + j * P))
            nt[m, j] = hi - lo
    K = np.maximum(1, (-(-nt // P)).max(axis=0)).astype(np.int64)

    # one gather per block: a multi-block gather's descriptor footprint
    # overflows the fixed 16 KiB HW SWDGE ring and stalls Q7 per instruction.
    # Process blocks largest-first so the post-last-gather tail is minimal.
    perm = [int(j) for j in np.argsort(-K, kind="stable")]
    pairs = [(j,) for j in perm]

    # column layout follows processing (perm) order
    Kp = [int(K[j]) for j in perm]
    C = np.concatenate([[0], np.cumsum(Kp)]).astype(np.int64)
    tot_k = int(C[-1])
    kmax = int(max(Kp))
    cstart = {j: int(C[i]) for i, j in enumerate(perm)}

    idx16 = np.zeros((n_cores, P, tot_k * 8), np.int16)
    dstcol_e = np.full((n_cores, P, tot_k), -1.0, np.float16)
    dstcol_o = np.full((n_cores, P, tot_k), -1.0, np.float16)
    wcol = np.zeros((n_cores, P, tot_k), np.float16)
    for m in range(n_cores):
        for i, j in enumerate(perm):
            s, d_ = blk_edges[m][j]
            kj = int(K[j])
            fi = np.full(kj * P, zero_pair, np.int64)
            fde = np.full(kj * P, -1.0, np.float32)
            fdo = np.full(kj * P, -1.0, np.float32)
            fw = np.zeros(kj * P, np.float32)
            n_ = len(s)
            fi[:n_] = s // 2
            ev = (s % 2 == 0)
            fde[:n_] = np.where(ev, d_, -1.0)
            fdo[:n_] = np.where(ev, -1.0, d_)
            fw[:n_] = dgo_val[s]
            c0 = int(C[i])
            blk16 = fi.astype(np.int16).reshape(-1, 16).T
            idx16[m, :, c0 * 8 : c0 * 8 + kj * 8] = np.tile(blk16, (8, 1))
            dstcol_e[m, :, c0 : c0 + kj] = fde.reshape(kj, P).T.astype(np.float16)
            dstcol_o[m, :, c0 : c0 + kj] = fdo.reshape(kj, P).T.astype(np.float16)
            wcol[m, :, c0 : c0 + kj] = fw.reshape(kj, P).T.astype(np.float16)

    # per-core rsqrt(deg_in) in block layout: din_blk[p, j] = node j*P + p
    din_blk = np.ones((n_cores, P, nblk), np.float32)
    for m in range(n_cores):
        base = m * npc
        v = np.ones(nblk * P, np.float32)
        v[:npc] = din_val[base : base + npc]
        din_blk[m] = v.reshape(nblk, P).T

    iota = np.tile(np.arange(P, dtype=np.float16), (P, 1))
    ident = np.eye(D, dtype=np.float32)

    in_maps = []
    for m in range(n_cores):
        base = m * npc
        in_maps.append(
            {
                "feats16_pad": feats16_pad,
                "feats_mine": np.ascontiguousarray(feats[base : base + npc]).astype(np.float32),
                "idx16": np.ascontiguousarray(idx16[m]),
                "dstcol_e": np.ascontiguousarray(dstcol_e[m]),
                "dstcol_o": np.ascontiguousarray(dstcol_o[m]),
                "wcol": np.ascontiguousarray(wcol[m]),
                "din_blk": np.ascontiguousarray(din_blk[m]),
                "Wmat": np.asarray(W, np.float32),
                "bvec": np.asarray(b, np.float32),
                "gamma": np.asarray(gamma, np.float32),
                "beta": np.asarray(beta, np.float32),
                "iota": iota,
                "ident": ident,
            }
        )

    meta = dict(
        N=N, D=D, n_cores=n_cores, npc=npc, nblk=nblk, rows_pp=rows_pp,
        npad=npad, npair=npair, K=[int(k) for k in K],
        Kp=Kp, cstart=cstart, pairs=pairs, tot_k=tot_k, kmax=kmax,
        skip_b=bool(np.all(np.asarray(b) == 0)),
        skip_gamma=bool(np.all(np.asarray(gamma) == 1)),
        skip_beta=bool(np.all(np.asarray(beta) == 0)),
    )
    return in_maps, meta


def _split_multiwaits(nc, mybir):
    """This walrus build allows only one sync-wait per instruction; hoist
    extra waits onto same-engine NoOps placed just before the instruction."""
    n = 0
    for f in nc.m.functions:
        for bb in f.blocks:
            newlist = []
            for inst in bb.instructions:
                si = getattr(inst, "sync_info", None)
                if si is not None and len(si.on_wait) > 1:
                    waits = list(si.on_wait)
                    for w in waits[:-1]:
                        nop = mybir.InstNoOp(name=f"I-WS-{n}", ins=[], outs=[])
                        n += 1
                        nop.engine = inst.engine
                        nop.sync_info = mybir.SyncInfo(on_wait=[w], on_update=[])
                        newlist.append(nop)
                    inst.sync_info = mybir.SyncInfo(
                        on_wait=[waits[-1]], on_update=list(si.on_update)
                    )
                newlist.append(inst)
            bb.instructions = newlist


# ---------------------------------------------------------------- device program
def build_nc(meta, debug=False, split_waits=True):
    _ensure_path()
    from contextlib import ExitStack

    import concourse.bass as bass
    import concourse.tile as tile
    from concourse import library_config, mybir

    dt = mybir.dt
    f32, f16, i16 = dt.float32, dt.float16, dt.int16
    Alu = mybir.AluOpType
    Act = mybir.ActivationFunctionType

    N = meta["N"]
    D = meta["D"]
    npc = meta["npc"]
    nblk = meta["nblk"]
    npad = meta["npad"]
    npair = meta["npair"]
    K = meta["K"]
    cstart = meta["cstart"]
    pairs = meta["pairs"]
    tot_k = meta["tot_k"]
    kmax = meta["kmax"]

    nc = bass.Bass()

    feats16_pad = nc.declare_dram_parameter("feats16_pad", [npad, D], f16, isOutput=False)
    feats_mine = nc.declare_dram_parameter("feats_mine", [npc, D], f32, isOutput=False)
    idx16_in = nc.declare_dram_parameter("idx16", [P, tot_k * 8], i16, isOutput=False)
    dstcol_e_in = nc.declare_dram_parameter("dstcol_e", [P, tot_k], f16, isOutput=False)
    dstcol_o_in = nc.declare_dram_parameter("dstcol_o", [P, tot_k], f16, isOutput=False)
    wcol_in = nc.declare_dram_parameter("wcol", [P, tot_k], f16, isOutput=False)
    din_in = nc.declare_dram_parameter("din_blk", [P, nblk], f32, isOutput=False)
    W_in = nc.declare_dram_parameter("Wmat", [D, D], f32, isOutput=False)
    b_in = nc.declare_dram_parameter("bvec", [D], f32, isOutput=False)
    gamma_in = nc.declare_dram_parameter("gamma", [D], f32, isOutput=False)
    beta_in = nc.declare_dram_parameter("beta", [D], f32, isOutput=False)
    iota_in = nc.declare_dram_parameter("iota", [P, P], f16, isOutput=False)
    ident_in = nc.declare_dram_parameter("ident", [D, D], f32, isOutput=False)
    out_t = nc.declare_dram_parameter("out", [npc, D], f32, isOutput=True)

    def bcast_row(ap, parts):
        return bass.AP(tensor=ap.tensor, offset=ap.offset, ap=[[0, parts]] + list(ap.ap))

    def bcast_mid(ap, reps):
        return bass.AP(tensor=ap.tensor, offset=ap.offset,
                       ap=[ap.ap[0], [0, reps], ap.ap[1]])

    def bcast_inner(ap, reps):
        return bass.AP(tensor=ap.tensor, offset=ap.offset,
                       ap=[ap.ap[0], ap.ap[1], [0, reps]])

    # features viewed as pair rows: [npair, 2*D] fp16 (256B rows)
    pair_view = bass.AP(tensor=feats16_pad[:].tensor, offset=0,
                        ap=[[2 * D, npair], [1, 2 * D]])

    with ExitStack() as ctx:
        tc = ctx.enter_context(tile.TileContext(nc))
        const = ctx.enter_context(tc.tile_pool(name="const", bufs=1))
        gp = ctx.enter_context(tc.tile_pool(name="gp", bufs=4))
        indp = ctx.enter_context(tc.tile_pool(name="indp", bufs=5))
        ep = ctx.enter_context(tc.tile_pool(name="ep", bufs=3))
        pp = ctx.enter_context(tc.tile_pool(name="pp", bufs=2, space="PSUM"))

        nc.gpsimd.load_library(library_config.mlp)

        # one shared register per distinct per-pair index count
        pair_nidx = {}
        for pr in pairs:
            n_idx = sum(K[j] for j in pr) * P
            if n_idx not in pair_nidx:
                pair_nidx[n_idx] = nc.gpsimd.to_reg(n_idx)

        # ---- constants -------------------------------------------------
        # the first gather depends only on this small index slice: load it
        # before everything else so gathering starts as early as possible
        k0 = K[pairs[0][0]]
        idx16_first = const.tile([P, k0 * 8], i16)
        nc.sync.dma_start(out=idx16_first[:], in_=idx16_in[:, : k0 * 8])

        iota_sb = const.tile([P, P], f16)
        nc.sync.dma_start(out=iota_sb[:], in_=iota_in[:])
        ident_sb = const.tile([D, D], f32)
        nc.sync.dma_start(out=ident_sb[:], in_=ident_in[:])
        w_f32 = const.tile([D, D], f32)
        nc.sync.dma_start(out=w_f32[:], in_=W_in[:])
        w_sb = const.tile([D, D], f16)
        nc.vector.tensor_copy(out=w_sb[:], in_=w_f32[:])
        b_bc = const.tile([P, D], f32)
        nc.sync.dma_start(out=b_bc[:], in_=bcast_row(b_in[:], P))
        gamma_bc = const.tile([P, D], f32)
        nc.sync.dma_start(out=gamma_bc[:], in_=bcast_row(gamma_in[:], P))
        beta_bc = const.tile([P, D], f32)
        nc.sync.dma_start(out=beta_bc[:], in_=bcast_row(beta_in[:], P))
        eps_sb = const.tile([P, 1], f32)
        nc.vector.memset(eps_sb[:], LN_EPS)
        idx16_sb = const.tile([P, (tot_k - k0) * 8], i16)
        nc.sync.dma_start(out=idx16_sb[:], in_=idx16_in[:, k0 * 8 :])
        dstcol_e_sb = const.tile([P, tot_k], f16)
        nc.sync.dma_start(out=dstcol_e_sb[:], in_=dstcol_e_in[:])
        dstcol_o_sb = const.tile([P, tot_k], f16)
        nc.sync.dma_start(out=dstcol_o_sb[:], in_=dstcol_o_in[:])
        wcol_sb = const.tile([P, tot_k], f16)
        nc.sync.dma_start(out=wcol_sb[:], in_=wcol_in[:])
        din_sb = const.tile([P, nblk], f32)
        nc.sync.dma_start(out=din_sb[:], in_=din_in[:])

        # ---- per-pair batched gather + per-block aggregation -----------
        kmax2 = max(sum(K[j] for j in pr) for pr in pairs)
        for ip, pr in enumerate(pairs):
            kpair = sum(K[j] for j in pr)
            c0 = cstart[pr[0]]
            if ip == 0:
                idx_src = idx16_first[:, : kpair * 8]
            else:
                idx_src = idx16_sb[:, (c0 - k0) * 8 : (c0 - k0 + kpair) * 8]

            g_blk = gp.tile([P, kmax2, 2 * D], f16, tag="g")
            nc.gpsimd.dma_gather(
                out_ap=g_blk[:, :kpair, :],
                in_ap=pair_view,
                idxs_ap=idx_src,
                num_idxs=kpair * P,
                num_idxs_reg=pair_nidx[kpair * P],
                elem_size=2 * D,
                single_packet=False,
            )

            for j in pr:
                kj = K[j]
                cj = cstart[j]
                goff = cj - c0
                bs = min(P, npc - j * P)

                ind2 = indp.tile([P, kmax, 2 * P], f16, tag="ind2")
                nc.vector.tensor_tensor(
                    out=ind2[:, :kj, 0:P],
                    in0=bcast_mid(iota_sb[:], kj),
                    in1=bcast_inner(dstcol_e_sb[:, cj : cj + kj], P),
                    op=Alu.is_equal,
                )
                nc.vector.tensor_tensor(
                    out=ind2[:, :kj, P : 2 * P],
                    in0=bcast_mid(iota_sb[:], kj),
                    in1=bcast_inner(dstcol_o_sb[:, cj : cj + kj], P),
                    op=Alu.is_equal,
                )
                nc.vector.tensor_tensor(
                    out=ind2[:, :kj, :],
                    in0=ind2[:, :kj, :],
                    in1=bcast_inner(wcol_sb[:, cj : cj + kj], 2 * P),
                    op=Alu.mult,
                )

                agg_ps = pp.tile([D, P], f32, tag="agg")
                for k in range(kj):
                    nc.tensor.matmul(
                        out=agg_ps[:],
                        lhsT=g_blk[:, goff + k, 0:D],
                        rhs=ind2[:, k, 0:P],
                        start=(k == 0),
                        stop=False,
                    )
                    nc.tensor.matmul(
                        out=agg_ps[:],
                        lhsT=g_blk[:, goff + k, D : 2 * D],
                        rhs=ind2[:, k, P : 2 * P],
                        start=False,
                        stop=(k == kj - 1),
                    )

                agg_sb = ep.tile([D, P], f16, tag="aggsb")
                nc.vector.tensor_copy(out=agg_sb[:], in_=agg_ps[:])
                w_ps = pp.tile([D, P], f32, tag="wps")
                nc.tensor.matmul(out=w_ps[:], lhsT=w_sb[:], rhs=agg_sb[:], start=True, stop=True)
                w_sbuf = ep.tile([D, P], f32, tag="wsb")
                nc.vector.tensor_copy(out=w_sbuf[:], in_=w_ps[:])
                t_ps = pp.tile([P, D], f32, tag="tps")
                nc.tensor.transpose(out=t_ps[:], in_=w_sbuf[:], identity=ident_sb[:])

                x = ep.tile([P, D], f32, tag="x")
                nc.vector.tensor_scalar(
                    out=x[:], in0=t_ps[:], scalar1=din_sb[:, j : j + 1], scalar2=None,
                    op0=Alu.mult,
                )
                if not meta.get("skip_b", False):
                    nc.vector.tensor_tensor(out=x[:], in0=x[:], in1=b_bc[:], op=Alu.add)
                nc.scalar.activation(out=x[:], in_=x[:], func=Act.Relu)
                f = ep.tile([P, D], f32, tag="f")
                nc.sync.dma_start(out=f[:bs, :], in_=feats_mine[j * P : j * P + bs, :])
                nc.vector.tensor_tensor(out=x[:bs, :], in0=x[:bs, :], in1=f[:bs, :], op=Alu.add)
                stats = ep.tile([P, 6], f32, tag="st")
                nc.vector.bn_stats(out=stats[:bs, :], in_=x[:bs, :])
                mv = ep.tile([P, 2], f32, tag="mv")
                nc.vector.bn_aggr(out=mv[:bs, :], in_=stats[:bs, :])
                sd = ep.tile([P, 1], f32, tag="sd")
                nc.scalar.activation(
                    out=sd[:bs, :], in_=mv[:bs, 1:2], func=Act.Sqrt, bias=eps_sb[:bs, :]
                )
                nc.vector.reciprocal(out=sd[:bs, :], in_=sd[:bs, :])
                y = ep.tile([P, D], f32, tag="y")
                nc.vector.tensor_scalar(
                    out=y[:bs, :],
                    in0=x[:bs, :],
                    scalar1=mv[:bs, 0:1],
                    scalar2=sd[:bs, :],
                    op0=Alu.subtract,
                    op1=Alu.mult,
                )
                if not meta.get("skip_gamma", False):
                    nc.vector.tensor_tensor(out=y[:bs, :], in0=y[:bs, :], in1=gamma_bc[:bs, :], op=Alu.mult)
                if not meta.get("skip_beta", False):
                    nc.vector.tensor_tensor(out=y[:bs, :], in0=y[:bs, :], in1=beta_bc[:bs, :], op=Alu.add)
                nc.sync.dma_start(out=out_t[j * P : j * P + bs, :], in_=y[:bs, :])

    # walrus requires the 16-word PSEUDO_INST encoding on the library-reload
    # pseudo instruction; bass leaves instr empty, so pack it here.
    import concourse.bass_isa as bass_isa

    for f in nc.m.functions:
        for bb in f.blocks:
            for inst in bb.instructions:
                if isinstance(inst, bass_isa.InstPseudoReloadLibraryIndex):
                    words, _ = bass_isa.isa_struct(
                        nc.isa,
                        nc.isa.Opcode.NEURON_ISA_TPB_OPCODE_PSEUDO_INST,
                        {"pseudo_opcode": 2, "lib_index": inst.lib_index},
                        struct_name="NEURON_ISA_TPB_PSEUDO_LIBRARY_RELOAD_INDEX_STRUCT",
                    )
                    inst.instr = words

    if split_waits:
        _split_multiwaits(nc, mybir)
    return nc


# ---------------------------------------------------------------- entry point
def kernel(feats, src, dst, W, b, gamma, beta):
    import os

    # recover cleanly if a previous run left the device in a bad state
    os.environ.setdefault("NEURON_RT_RESET_CORES", "1")
    _ensure_path()
    from concourse.bass_utils import run_bass_kernel_spmd

    n_cores = 8
    feats = np.asarray(feats, np.float32)
    in_maps, meta = host_prep(feats, src, dst, W, b, gamma, beta, n_cores)
    nc = build_nc(meta)
    res = run_bass_kernel_spmd(nc, in_maps, core_ids=list(range(n_cores)))
    out = np.concatenate([r["out"] for r in res.results], axis=0)
    return out[: meta["N"]].astype(np.float32)


# revision 29
# speedup vs baseline: 1.1324x; 1.1324x over previous
"""GCN layer (DGL GraphConv norm='both' + relu + residual + LayerNorm) on 8 trn2 NeuronCores.

Final: gathers run directly on an fp16 cast of the node features (no device-side
table build), with the src-degree norm folded into the one-hot scatter masks.
One batched gather per dst block via gpsimd.dma_gather (Q7 'mlp' library) on a
pair-row view [npair, 128] so indices fit int16 (idx = src//2). Each 128-edge
tile runs two masked matmuls — lhsT slices the even/odd half of the gathered
pair row against per-parity masks ind_e/ind_o — so no parity sorting is needed
and the tile schedule is minimal (K[j] = max over cores of ceil(edges/128)).
Dst-degree norm vectors are precomputed host-side. The kernel is bound by Q7
descriptor generation at ~7.6ns per gathered row; everything else hides under it.
"""

import numpy as np


def _ensure_path():
    try:
        import concourse  # noqa: F401
    except ImportError:
        import sys

        for p in ("/opt/trn_rl_repo", "/root/.axon_site/_ro/trn_rl_repo"):
            if p not in sys.path:
                sys.path.insert(0, p)


P = 128
LN_EPS = 1e-5


# ---------------------------------------------------------------- host prep
def host_prep(feats, src, dst, W, b, gamma, beta, n_cores):
    N, D = feats.shape
    assert N % n_cores == 0 and N % 2 == 0
    npc = N // n_cores                      # nodes per core
    nblk = (npc + P - 1) // P               # 128-node blocks per core
    rows_pp = (N + 2 + P - 1) // P          # table rows per partition
    npad = rows_pp * P                      # padded table rows (even, >= N+2)
    npair = npad // 2
    zero_pair = N // 2                      # nodes N, N+1 are zero rows

    src = np.asarray(src).astype(np.int64)
    dst = np.asarray(dst).astype(np.int64)

    feats16_pad = np.zeros((npad, D), np.float16)
    feats16_pad[:N] = feats.astype(np.float16)

    order = np.argsort(dst, kind="stable")
    src_s = src[order]
    dst_s = dst[order]
    rp_dst = np.searchsorted(dst_s, np.arange(N + 1)).astype(np.int64)

    # host-side degree vectors
    deg_out = np.bincount(src, minlength=N).astype(np.float64)
    deg_in = np.bincount(dst, minlength=N).astype(np.float64)
    dgo_val = (1.0 / np.sqrt(np.clip(deg_out, 1.0, None))).astype(np.float32)
    din_val = (1.0 / np.sqrt(np.clip(deg_in, 1.0, None))).astype(np.float32)

    # ---- per (core, block) edge lists (no parity sort: each tile runs two
    # masked matmuls, one per src-parity half of the gathered pair) ---------
    nt = np.zeros((n_cores, nblk), np.int64)
    blk_edges = [[None] * nblk for _ in range(n_cores)]
    for m in range(n_cores):
        base = m * npc
        for j in range(nblk):
            lo = rp_dst[base + min(j * P, npc)]
            hi = rp_dst[base + min((j + 1) * P, npc)]
            blk_edges[m][j] = (src_s[lo:hi], dst_s[lo:hi] - (base + j * P))
            nt[m, j] = hi - lo
    K = np.maximum(1, (-(-nt // P)).max(axis=0)).astype(np.int64)

    # one gather per block: a multi-block gather's descriptor footprint
    # overflows the fixed 16 KiB HW SWDGE ring and stalls Q7 per instruction.
    # Process blocks largest-first so the post-last-gather tail is minimal.
    perm = [int(j) for j in np.argsort(-K, kind="stable")]
    pairs = [(j,) for j in perm]

    # column layout follows processing (perm) order
    Kp = [int(K[j]) for j in perm]
    C = np.concatenate([[0], np.cumsum(Kp)]).astype(np.int64)
    tot_k = int(C[-1])
    kmax = int(max(Kp))
    cstart = {j: int(C[i]) for i, j in enumerate(perm)}

    idx16 = np.zeros((n_cores, P, tot_k * 8), np.int16)
    dstcol_e = np.full((n_cores, P, tot_k), -1.0, np.float16)
    dstcol_o = np.full((n_cores, P, tot_k), -1.0, np.float16)
    wcol = np.zeros((n_cores, P, tot_k), np.float16)
    for m in range(n_cores):
        for i, j in enumerate(perm):
            s, d_ = blk_edges[m][j]
            kj = int(K[j])
            fi = np.full(kj * P, zero_pair, np.int64)
            fde = np.full(kj * P, -1.0, np.float32)
            fdo = np.full(kj * P, -1.0, np.float32)
            fw = np.zeros(kj * P, np.float32)
            n_ = len(s)
            fi[:n_] = s // 2
            ev = (s % 2 == 0)
            fde[:n_] = np.where(ev, d_, -1.0)
            fdo[:n_] = np.where(ev, -1.0, d_)
            fw[:n_] = dgo_val[s]
            c0 = int(C[i])
            blk16 = fi.astype(np.int16).reshape(-1, 16).T
            idx16[m, :, c0 * 8 : c0 * 8 + kj * 8] = np.tile(blk16, (8, 1))
            dstcol_e[m, :, c0 : c0 + kj] = fde.reshape(kj, P).T.astype(np.float16)
            dstcol_o[m, :, c0 : c0 + kj] = fdo.reshape(kj, P).T.astype(np.float16)
            wcol[m, :, c0 : c0 + kj] = fw.reshape(kj, P).T.astype(np.float16)

    # per-core rsqrt(deg_in) in block layout: din_blk[p, j] = node j*P + p
    din_blk = np.ones((n_cores, P, nblk), np.float32)
    for m in range(n_cores):
        base = m * npc
        v = np.ones(nblk * P, np.float32)
        v[:npc] = din_val[base : base + npc]
        din_blk[m] = v.reshape(nblk, P).T

    iota = np.tile(np.arange(P, dtype=np.float16), (P, 1))
    ident = np.eye(D, dtype=np.float32)

    in_maps = []
    for m in range(n_cores):
        base = m * npc
        in_maps.append(
            {
                "feats16_pad": feats16_pad,
                "feats_mine": np.ascontiguousarray(feats[base : base + npc]).astype(np.float32),
                "idx16": np.ascontiguousarray(idx16[m]),
                "dstcol_e": np.ascontiguousarray(dstcol_e[m]),
                "dstcol_o": np.ascontiguousarray(dstcol_o[m]),
                "wcol": np.ascontiguousarray(wcol[m]),
                "din_blk": np.ascontiguousarray(din_blk[m]),
                "Wmat": np.asarray(W, np.float32),
                "bvec": np.asarray(b, np.float32),
                "gamma": np.asarray(gamma, np.float32),
                "beta": np.asarray(beta, np.float32),
                "iota": iota,
                "ident": ident,
            }
        )

    meta = dict(
        N=N, D=D, n_cores=n_cores, npc=npc, nblk=nblk, rows_pp=rows_pp,
        npad=npad, npair=npair, K=[int(k) for k in K],
        Kp=Kp, cstart=cstart, pairs=pairs, tot_k=tot_k, kmax=kmax,
        skip_b=bool(np.all(np.asarray(b) == 0)),
        skip_gamma=bool(np.all(np.asarray(gamma) == 1)),
        skip_beta=bool(np.all(np.asarray(beta) == 0)),
    )
    return in_maps, meta


def _split_multiwaits(nc, mybir):
    """This walrus build allows only one sync-wait per instruction; hoist
    extra waits onto same-engine NoOps placed just before the instruction."""
    n = 0
    for f in nc.m.functions:
        for bb in f.blocks:
            newlist = []
            for inst in bb.instructions:
                si = getattr(inst, "sync_info", None)
                if si is not None and len(si.on_wait) > 1:
                    waits = list(si.on_wait)
                    for w in waits[:-1]:
                        nop = mybir.InstNoOp(name=f"I-WS-{n}", ins=[], outs=[])
                        n += 1
                        nop.engine = inst.engine
                        nop.sync_info = mybir.SyncInfo(on_wait=[w], on_update=[])
                        newlist.append(nop)
                    inst.sync_info = mybir.SyncInfo(
                        on_wait=[waits[-1]], on_update=list(si.on_update)
                    )
                newlist.append(inst)
            bb.instructions = newlist


# ---------------------------------------------------------------- device program
def build_nc(meta, debug=False, split_waits=True):
    _ensure_path()
    from contextlib import ExitStack

    import concourse.bass as bass
    import concourse.tile as tile
    from concourse import library_config, mybir

    dt = mybir.dt
    f32, f16, i16 = dt.float32, dt.float16, dt.int16
    Alu = mybir.AluOpType
    Act = mybir.ActivationFunctionType

    N = meta["N"]
    D = meta["D"]
    npc = meta["npc"]
    nblk = meta["nblk"]
    npad = meta["npad"]
    npair = meta["npair"]
    K = meta["K"]
    cstart = meta["cstart"]
    pairs = meta["pairs"]
    tot_k = meta["tot_k"]
    kmax = meta["kmax"]

    nc = bass.Bass()

    feats16_pad = nc.declare_dram_parameter("feats16_pad", [npad, D], f16, isOutput=False)
    feats_mine = nc.declare_dram_parameter("feats_mine", [npc, D], f32, isOutput=False)
    idx16_in = nc.declare_dram_parameter("idx16", [P, tot_k * 8], i16, isOutput=False)
    dstcol_e_in = nc.declare_dram_parameter("dstcol_e", [P, tot_k], f16, isOutput=False)
    dstcol_o_in = nc.declare_dram_parameter("dstcol_o", [P, tot_k], f16, isOutput=False)
    wcol_in = nc.declare_dram_parameter("wcol", [P, tot_k], f16, isOutput=False)
    din_in = nc.declare_dram_parameter("din_blk", [P, nblk], f32, isOutput=False)
    W_in = nc.declare_dram_parameter("Wmat", [D, D], f32, isOutput=False)
    b_in = nc.declare_dram_parameter("bvec", [D], f32, isOutput=False)
    gamma_in = nc.declare_dram_parameter("gamma", [D], f32, isOutput=False)
    beta_in = nc.declare_dram_parameter("beta", [D], f32, isOutput=False)
    iota_in = nc.declare_dram_parameter("iota", [P, P], f16, isOutput=False)
    ident_in = nc.declare_dram_parameter("ident", [D, D], f32, isOutput=False)
    out_t = nc.declare_dram_parameter("out", [npc, D], f32, isOutput=True)

    def bcast_row(ap, parts):
        return bass.AP(tensor=ap.tensor, offset=ap.offset, ap=[[0, parts]] + list(ap.ap))

    def bcast_mid(ap, reps):
        return bass.AP(tensor=ap.tensor, offset=ap.offset,
                       ap=[ap.ap[0], [0, reps], ap.ap[1]])

    def bcast_inner(ap, reps):
        return bass.AP(tensor=ap.tensor, offset=ap.offset,
                       ap=[ap.ap[0], ap.ap[1], [0, reps]])

    # features viewed as pair rows: [npair, 2*D] fp16 (256B rows)
    pair_view = bass.AP(tensor=feats16_pad[:].tensor, offset=0,
                        ap=[[2 * D, npair], [1, 2 * D]])

    with ExitStack() as ctx:
        tc = ctx.enter_context(tile.TileContext(nc))
        const = ctx.enter_context(tc.tile_pool(name="const", bufs=1))
        gp = ctx.enter_context(tc.tile_pool(name="gp", bufs=3))
        indp = ctx.enter_context(tc.tile_pool(name="indp", bufs=4))
        ep = ctx.enter_context(tc.tile_pool(name="ep", bufs=3))
        pp = ctx.enter_context(tc.tile_pool(name="pp", bufs=2, space="PSUM"))

        nc.gpsimd.load_library(library_config.mlp)

        # one shared register per distinct per-pair index count
        pair_nidx = {}
        for pr in pairs:
            n_idx = sum(K[j] for j in pr) * P
            if n_idx not in pair_nidx:
                pair_nidx[n_idx] = nc.gpsimd.to_reg(n_idx)

        # ---- constants -------------------------------------------------
        # the first gather depends only on this small index slice: load it
        # before everything else so gathering starts as early as possible
        k0 = K[pairs[0][0]]
        idx16_first = const.tile([P, k0 * 8], i16)
        nc.sync.dma_start(out=idx16_first[:], in_=idx16_in[:, : k0 * 8])

        iota_sb = const.tile([P, P], f16)
        nc.sync.dma_start(out=iota_sb[:], in_=iota_in[:])
        ident_sb = const.tile([D, D], f32)
        nc.sync.dma_start(out=ident_sb[:], in_=ident_in[:])
        w_f32 = const.tile([D, D], f32)
        nc.sync.dma_start(out=w_f32[:], in_=W_in[:])
        w_sb = const.tile([D, D], f16)
        nc.vector.tensor_copy(out=w_sb[:], in_=w_f32[:])
        b_bc = const.tile([P, D], f32)
        nc.sync.dma_start(out=b_bc[:], in_=bcast_row(b_in[:], P))
        gamma_bc = const.tile([P, D], f32)
        nc.sync.dma_start(out=gamma_bc[:], in_=bcast_row(gamma_in[:], P))
        beta_bc = const.tile([P, D], f32)
        nc.sync.dma_start(out=beta_bc[:], in_=bcast_row(beta_in[:], P))
        eps_sb = const.tile([P, 1], f32)
        nc.vector.memset(eps_sb[:], LN_EPS)
        idx16_sb = const.tile([P, (tot_k - k0) * 8], i16)
        nc.sync.dma_start(out=idx16_sb[:], in_=idx16_in[:, k0 * 8 :])
        dstcol_e_sb = const.tile([P, tot_k], f16)
        nc.sync.dma_start(out=dstcol_e_sb[:], in_=dstcol_e_in[:])
        dstcol_o_sb = const.tile([P, tot_k], f16)
        nc.sync.dma_start(out=dstcol_o_sb[:], in_=dstcol_o_in[:])
        wcol_sb = const.tile([P, tot_k], f16)
        nc.sync.dma_start(out=wcol_sb[:], in_=wcol_in[:])
        din_sb = const.tile([P, nblk], f32)
        nc.sync.dma_start(out=din_sb[:], in_=din_in[:])

        # ---- per-pair batched gather + per-block aggregation -----------
        kmax2 = max(sum(K[j] for j in pr) for pr in pairs)
        for ip, pr in enumerate(pairs):
            kpair = sum(K[j] for j in pr)
            c0 = cstart[pr[0]]
            if ip == 0:
                idx_src = idx16_first[:, : kpair * 8]
            else:
                idx_src = idx16_sb[:, (c0 - k0) * 8 : (c0 - k0 + kpair) * 8]

            g_blk = gp.tile([P, kmax2, 2 * D], f16, tag="g")
            nc.gpsimd.dma_gather(
                out_ap=g_blk[:, :kpair, :],
                in_ap=pair_view,
                idxs_ap=idx_src,
                num_idxs=kpair * P,
                num_idxs_reg=pair_nidx[kpair * P],
                elem_size=2 * D,
                single_packet=False,
            )

            for j in pr:
                kj = K[j]
                cj = cstart[j]
                goff = cj - c0
                bs = min(P, npc - j * P)

                ind2 = indp.tile([P, kmax, 2 * P], f16, tag="ind2")
                nc.vector.tensor_tensor(
                    out=ind2[:, :kj, 0:P],
                    in0=bcast_mid(iota_sb[:], kj),
                    in1=bcast_inner(dstcol_e_sb[:, cj : cj + kj], P),
                    op=Alu.is_equal,
                )
                nc.vector.tensor_tensor(
                    out=ind2[:, :kj, P : 2 * P],
                    in0=bcast_mid(iota_sb[:], kj),
                    in1=bcast_inner(dstcol_o_sb[:, cj : cj + kj], P),
                    op=Alu.is_equal,
                )
                nc.vector.tensor_tensor(
                    out=ind2[:, :kj, :],
                    in0=ind2[:, :kj, :],
                    in1=bcast_inner(wcol_sb[:, cj : cj + kj], 2 * P),
                    op=Alu.mult,
                )

                agg_ps = pp.tile([D, P], f32, tag="agg")
                for k in range(kj):
                    nc.tensor.matmul(
                        out=agg_ps[:],
                        lhsT=g_blk[:, goff + k, 0:D],
                        rhs=ind2[:, k, 0:P],
                        start=(k == 0),
                        stop=False,
                    )
                    nc.tensor.matmul(
                        out=agg_ps[:],
                        lhsT=g_blk[:, goff + k, D : 2 * D],
                        rhs=ind2[:, k, P : 2 * P],
                        start=False,
                        stop=(k == kj - 1),
                    )

                agg_sb = ep.tile([D, P], f16, tag="aggsb")
                nc.vector.tensor_copy(out=agg_sb[:], in_=agg_ps[:])
                w_ps = pp.tile([D, P], f32, tag="wps")
                nc.tensor.matmul(out=w_ps[:], lhsT=w_sb[:], rhs=agg_sb[:], start=True, stop=True)
                w_sbuf = ep.tile([D, P], f32, tag="wsb")
                nc.vector.tensor_copy(out=w_sbuf[:], in_=w_ps[:])
                t_ps = pp.tile([P, D], f32, tag="tps")
                nc.tensor.transpose(out=t_ps[:], in_=w_sbuf[:], identity=ident_sb[:])

                x = ep.tile([P, D], f32, tag="x")
                nc.vector.tensor_scalar(
                    out=x[:], in0=t_ps[:], scalar1=din_sb[:, j : j + 1], scalar2=None,
                    op0=Alu.mult,
                )
                if not meta.get("skip_b", False):
                    nc.vector.tensor_tensor(out=x[:], in0=x[:], in1=b_bc[:], op=Alu.add)
                nc.scalar.activation(out=x[:], in_=x[:], func=Act.Relu)
                f = ep.tile([P, D], f32, tag="f")
                nc.sync.dma_start(out=f[:bs, :], in_=feats_mine[j * P : j * P + bs, :])
                nc.vector.tensor_tensor(out=x[:bs, :], in0=x[:bs, :], in1=f[:bs, :], op=Alu.add)
                stats = ep.tile([P, 6], f32, tag="st")
                nc.vector.bn_stats(out=stats[:bs, :], in_=x[:bs, :])
                mv = ep.tile([P, 2], f32, tag="mv")
                nc.vector.bn_aggr(out=mv[:bs, :], in_=stats[:bs, :])
                sd = ep.tile([P, 1], f32, tag="sd")
                nc.scalar.activation(
                    out=sd[:bs, :], in_=mv[:bs, 1:2], func=Act.Sqrt, bias=eps_sb[:bs, :]
                )
                nc.vector.reciprocal(out=sd[:bs, :], in_=sd[:bs, :])
                y = ep.tile([P, D], f32, tag="y")
                nc.vector.tensor_scalar(
                    out=y[:bs, :],
                    in0=x[:bs, :],
                    scalar1=mv[:bs, 0:1],
                    scalar2=sd[:bs, :],
                    op0=Alu.subtract,
                    op1=Alu.mult,
                )
                if not meta.get("skip_gamma", False):
                    nc.vector.tensor_tensor(out=y[:bs, :], in0=y[:bs, :], in1=gamma_bc[:bs, :], op=Alu.mult)
                if not meta.get("skip_beta", False):
                    nc.vector.tensor_tensor(out=y[:bs, :], in0=y[:bs, :], in1=beta_bc[:bs, :], op=Alu.add)
                nc.sync.dma_start(out=out_t[j * P : j * P + bs, :], in_=y[:bs, :])

    # walrus requires the 16-word PSEUDO_INST encoding on the library-reload
    # pseudo instruction; bass leaves instr empty, so pack it here.
    import concourse.bass_isa as bass_isa

    for f in nc.m.functions:
        for bb in f.blocks:
            for inst in bb.instructions:
                if isinstance(inst, bass_isa.InstPseudoReloadLibraryIndex):
                    words, _ = bass_isa.isa_struct(
                        nc.isa,
                        nc.isa.Opcode.NEURON_ISA_TPB_OPCODE_PSEUDO_INST,
                        {"pseudo_opcode": 2, "lib_index": inst.lib_index},
                        struct_name="NEURON_ISA_TPB_PSEUDO_LIBRARY_RELOAD_INDEX_STRUCT",
                    )
                    inst.instr = words

    if split_waits:
        _split_multiwaits(nc, mybir)
    return nc


# ---------------------------------------------------------------- entry point
def kernel(feats, src, dst, W, b, gamma, beta):
    import os

    # recover cleanly if a previous run left the device in a bad state
    os.environ.setdefault("NEURON_RT_RESET_CORES", "1")
    _ensure_path()
    from concourse.bass_utils import run_bass_kernel_spmd

    n_cores = 8
    feats = np.asarray(feats, np.float32)
    in_maps, meta = host_prep(feats, src, dst, W, b, gamma, beta, n_cores)
    nc = build_nc(meta)
    res = run_bass_kernel_spmd(nc, in_maps, core_ids=list(range(n_cores)))
    out = np.concatenate([r["out"] for r in res.results], axis=0)
    return out[: meta["N"]].astype(np.float32)
